# revision 12
# baseline (speedup 1.0000x reference)
"""Multi-head attention (B=2, S=2048, D=1024, H=16) on 8 TRN2 NeuronCores.

Sharding: (batch, head-group) — core c handles batch c//4 and heads
[4*(c%4), 4*(c%4)+4). Each core projects its batch's tokens onto its 4 heads'
slices of Wq/Wk/Wv (column shards), runs attention for those heads, and
multiplies by its row-shard of Wo, producing a partial [S, D] output. Host
sums the 4 partials per batch and adds bo.

Device layout notes:
  - Inputs are host-pre-transposed to feature-major X^T [D, S] so projection
    matmuls (contraction over D) read natural tiles.
  - Scores are computed transposed (S^T [key j, query i]) so the PV matmul
    consumes exp(S^T) directly with V as the stationary operand; a ones
    column appended to V yields the softmax denominator in the same matmul.
  - Normalization: DVE reciprocal of the denominator row + gpsimd
    partition_broadcast + DVE multiply on the 64-row ctx^T tile.
  - All matmuls run as float32r (full PE rate at N=512); PE transpose for V.
"""

import os
import numpy as np

S = 2048          # sequence length
D = 1024          # model dim
HPC = 4           # heads per core
DK = 64           # head dim
M = HPC * DK      # per-core projection width = 256
NC = 8            # cores
IW = 1024         # attention query-block width (free dim of exp / psum)

_cached = {}


def _build(debug=False):
    import concourse.bass as bass
    import concourse.bacc as bacc
    import concourse.tile as tile
    import concourse.mybir as mybir
    from contextlib import ExitStack

    f32 = mybir.dt.float32
    f32r = mybir.dt.float32r
    AF = mybir.ActivationFunctionType

    def r(ap):
        return ap.bitcast(f32r)

    nc = bacc.Bacc(
        "TRN2",
        target_bir_lowering=False,
        debug=False,
        enable_asserts=False,
        num_devices=NC,
    )

    # DRAM I/O (per-core shapes)
    xqT_d = nc.dram_tensor("xqT", [D, S], f32, kind="ExternalInput").ap()
    xkT_d = nc.dram_tensor("xkT", [D, S], f32, kind="ExternalInput").ap()
    xvT_d = nc.dram_tensor("xvT", [D, S], f32, kind="ExternalInput").ap()
    wq_d = nc.dram_tensor("wq", [D, M], f32, kind="ExternalInput").ap()
    wk_d = nc.dram_tensor("wk", [D, M], f32, kind="ExternalInput").ap()
    wv_d = nc.dram_tensor("wv", [D, M], f32, kind="ExternalInput").ap()
    wo_d = nc.dram_tensor("wo", [M, D], f32, kind="ExternalInput").ap()
    bq_d = nc.dram_tensor("bq", [M], f32, kind="ExternalInput").ap()
    bk_d = nc.dram_tensor("bk", [M], f32, kind="ExternalInput").ap()
    bv_d = nc.dram_tensor("bv", [M], f32, kind="ExternalInput").ap()
    ident_d = nc.dram_tensor("ident", [128, 128], f32, kind="ExternalInput").ap()
    out_d = nc.dram_tensor("out", [S, D], f32, kind="ExternalOutput").ap()
    if debug:
        dbg = {
            "qT": nc.dram_tensor("dbg_qT", [128, 2, S], f32, kind="ExternalOutput").ap(),
            "kT": nc.dram_tensor("dbg_kT", [128, 2, S], f32, kind="ExternalOutput").ap(),
            "v0": nc.dram_tensor("dbg_v0", [128, 16, 65], f32, kind="ExternalOutput").ap(),
            "e00": nc.dram_tensor("dbg_e00", [128, 1024], f32, kind="ExternalOutput").ap(),
            "den": nc.dram_tensor("dbg_den", [1, 1024], f32, kind="ExternalOutput").ap(),
            "inv": nc.dram_tensor("dbg_inv", [1, 1024], f32, kind="ExternalOutput").ap(),
            "bca": nc.dram_tensor("dbg_bca", [64, 1024], f32, kind="ExternalOutput").ap(),
            "c0": nc.dram_tensor("dbg_c0", [64, S], f32, kind="ExternalOutput").ap(),
        }

    NDC = D // 128   # 8 d-chunks of contraction
    NMC = M // 128   # 2 m-chunks of per-core projection width
    NSC = S // 512   # 4 s-chunks for projections
    NJC = S // 128   # 16 key chunks
    NIH = S // IW    # 2 query halves
    NIC = S // 128   # 16 query chunks for output proj

    with tile.TileContext(nc) as tc:
        with ExitStack() as outer:
            # ---- persistent pools ----
            wpool = outer.enter_context(tc.tile_pool(name="w", bufs=1))
            qkv = outer.enter_context(tc.tile_pool(name="qkv", bufs=1))
            vsbp = outer.enter_context(tc.tile_pool(name="vsb", bufs=1))
            ctxp = outer.enter_context(tc.tile_pool(name="ctx", bufs=1))

            wq_sb = wpool.tile([128, NDC, M], f32r, tag="wq")
            wk_sb = wpool.tile([128, NDC, M], f32r, tag="wk")
            wv_sb = wpool.tile([128, NDC, M], f32r, tag="wv")
            wo_sb = wpool.tile([64, HPC, D], f32r, tag="wo")
            bq_sb = wpool.tile([128, NMC], f32, tag="bq")
            bk_sb = wpool.tile([128, NMC], f32, tag="bk")
            bv_sb = wpool.tile([128, NMC], f32, tag="bv")
            ident = wpool.tile([128, 128], f32, tag="ident")

            w_r = lambda ap: ap.rearrange("(n p) m -> p n m", p=128)
            nc.sync.dma_start(out=wq_sb, in_=r(w_r(wq_d)))
            nc.sync.dma_start(out=wk_sb, in_=r(w_r(wk_d)))
            nc.sync.dma_start(out=wv_sb, in_=r(w_r(wv_d)))
            nc.sync.dma_start(out=wo_sb, in_=r(wo_d.rearrange("(h d) n -> d h n", d=64)))
            nc.sync.dma_start(out=bq_sb, in_=bq_d.rearrange("(n p) -> p n", p=128))
            nc.sync.dma_start(out=bk_sb, in_=bk_d.rearrange("(n p) -> p n", p=128))
            nc.sync.dma_start(out=bv_sb, in_=bv_d.rearrange("(n p) -> p n", p=128))
            nc.sync.dma_start(out=ident, in_=ident_d)

            # Q^T/K^T/V^T in SBUF: [m (within chunk), m-chunk, s]
            qT = qkv.tile([128, NMC, S], f32r, tag="qT")
            kT = qkv.tile([128, NMC, S], f32r, tag="kT")
            vT = qkv.tile([128, NMC, S], f32, tag="vT")
            # seq-major V with ones column, per head: [s (within chunk), j-chunk, 65]
            v_sb = [vsbp.tile([128, NJC, DK + 1], f32r, tag=f"v{h}", name=f"v{h}") for h in range(HPC)]
            for h in range(HPC):
                nc.scalar.activation(
                    out=v_sb[h][:, :, DK],
                    in_=ident[:, 0:NJC],
                    func=AF.Copy,
                    bias=1.0,
                    scale=0.0,
                )
            # per-head ctx^T [dk, s] (normalized), feeds output projection
            ctx_t = [ctxp.tile([64, S], f32r, tag=f"c{h}", name=f"c{h}") for h in range(HPC)]

            # ---- phase A: projections + V transpose ----
            with ExitStack() as ph_a:
                xt = ph_a.enter_context(tc.tile_pool(name="xt", bufs=NDC))
                pps = ph_a.enter_context(
                    tc.tile_pool(name="pps", bufs=3, space="PSUM")
                )
                vtp = ph_a.enter_context(
                    tc.tile_pool(name="vtp", bufs=2, space="PSUM")
                )

                for xdram, w_sb, b_sb, dst in (
                    (xkT_d, wk_sb, bk_sb, kT),
                    (xqT_d, wq_sb, bq_sb, qT),
                    (xvT_d, wv_sb, bv_sb, vT),
                ):
                    xts = []
                    for dc in range(NDC):
                        t = xt.tile([128, S], f32r, tag="x")
                        nc.sync.dma_start(
                            out=t, in_=r(xdram[dc * 128 : (dc + 1) * 128, :])
                        )
                        xts.append(t)
                    for mc in range(NMC):
                        for sc in range(NSC):
                            ps = pps.tile([128, 512], f32, tag="ps")
                            for dc in range(NDC):
                                nc.tensor.matmul(
                                    ps,
                                    lhsT=w_sb[:, dc, mc * 128 : (mc + 1) * 128],
                                    rhs=xts[dc][:, sc * 512 : (sc + 1) * 512],
                                    start=(dc == 0),
                                    stop=(dc == NDC - 1),
                                )
                            nc.scalar.add(
                                out=dst[:, mc, sc * 512 : (sc + 1) * 512],
                                in_=ps,
                                add=b_sb[:, mc : mc + 1],
                            )

                # V: transpose to seq-major per head
                for mc in range(NMC):
                    for sb in range(NJC):
                        tp = vtp.tile([128, 128], f32, tag="tp")
                        nc.tensor.transpose(
                            tp,
                            in_=vT[:, mc, sb * 128 : (sb + 1) * 128],
                            identity=ident,
                        )
                        nc.vector.tensor_copy(
                            out=v_sb[2 * mc][:, sb, 0:DK], in_=tp[:, 0:DK]
                        )
                        nc.vector.tensor_copy(
                            out=v_sb[2 * mc + 1][:, sb, 0:DK], in_=tp[:, DK:128]
                        )

            # ---- phase B: attention per head ----
            with ExitStack() as ph_b:
                qkp = ph_b.enter_context(
                    tc.tile_pool(name="qkp", bufs=2, space="PSUM")
                )
                pvp = ph_b.enter_context(
                    tc.tile_pool(name="pvp", bufs=2, space="PSUM")
                )
                ep = ph_b.enter_context(tc.tile_pool(name="ep", bufs=4))
                sm = ph_b.enter_context(tc.tile_pool(name="sm", bufs=4))

                for h in range(HPC):
                    mc, off = divmod(h, 2)
                    off *= 64
                    for ih in range(NIH):
                        pv = pvp.tile([128, IW], f32, tag="pv")
                        for jc in range(NJC):
                            qk = qkp.tile([128, IW], f32, tag="qk")
                            for ha in range(IW // 512):
                                i0 = ih * IW + ha * 512
                                nc.tensor.matmul(
                                    qk[:, ha * 512 : (ha + 1) * 512],
                                    lhsT=kT[
                                        off : off + DK,
                                        mc,
                                        jc * 128 : (jc + 1) * 128,
                                    ],
                                    rhs=qT[off : off + DK, mc, i0 : i0 + 512],
                                    start=True,
                                    stop=True,
                                )
                            e = ep.tile([128, IW], f32r, tag="e")
                            nc.scalar.activation(
                                out=e, in_=qk, func=AF.Exp, scale=1.0 / np.sqrt(DK)
                            )
                            if debug and h == 0 and ih == 0 and jc == 0:
                                nc.sync.dma_start(out=dbg["e00"], in_=e.bitcast(f32))
                            for ha in range(IW // 512):
                                nc.tensor.matmul(
                                    pv[0 : DK + 1, ha * 512 : (ha + 1) * 512],
                                    lhsT=v_sb[h][:, jc, :],
                                    rhs=e[:, ha * 512 : (ha + 1) * 512],
                                    start=(jc == 0),
                                    stop=(jc == NJC - 1),
                                )
                        inv = sm.tile([65, IW], f32, tag="inv")
                        nc.vector.reciprocal(
                            out=inv[64:65, :], in_=pv[64:65, :]
                        )
                        invp0 = sm.tile([1, IW], f32, tag="invp0")
                        nc.sync.dma_start(out=invp0, in_=inv[64:65, :])
                        bca = sm.tile([64, IW], f32, tag="bca")
                        nc.gpsimd.partition_broadcast(bca, invp0)
                        nc.vector.tensor_mul(
                            ctx_t[h][:, ih * IW : (ih + 1) * IW],
                            pv[0:DK, :],
                            bca,
                        )
                        if debug and h == 0 and ih == 0:
                            nc.sync.dma_start(out=dbg["inv"], in_=inv[64:65, :])
                            nc.sync.dma_start(out=dbg["bca"], in_=bca)

            # ---- phase C: output projection ----
            with ExitStack() as ph_c:
                ops = ph_c.enter_context(
                    tc.tile_pool(name="ops", bufs=4, space="PSUM")
                )
                ost = ph_c.enter_context(tc.tile_pool(name="ost", bufs=4))
                for ic in range(NIC):
                    for nh in range(2):
                        ps = ops.tile([128, 512], f32, tag="ops")
                        for h in range(HPC):
                            nc.tensor.matmul(
                                ps,
                                lhsT=ctx_t[h][:, ic * 128 : (ic + 1) * 128],
                                rhs=wo_sb[:, h, nh * 512 : (nh + 1) * 512],
                                start=(h == 0),
                                stop=(h == HPC - 1),
                            )
                        st = ost.tile([128, 512], f32, tag="ost")
                        nc.scalar.copy(out=st, in_=ps)
                        nc.sync.dma_start(
                            out=out_d[
                                ic * 128 : (ic + 1) * 128, nh * 512 : (nh + 1) * 512
                            ],
                            in_=st,
                        )

            if debug:
                nc.sync.dma_start(out=dbg["qT"], in_=qT.bitcast(f32))
                nc.sync.dma_start(out=dbg["kT"], in_=kT.bitcast(f32))
                nc.sync.dma_start(out=dbg["v0"], in_=v_sb[0].bitcast(f32))
                nc.sync.dma_start(out=dbg["c0"], in_=ctx_t[0].bitcast(f32))

    nc.compile()
    return nc


def _get_nc(debug=False):
    key = ("nc", debug)
    if key not in _cached:
        _cached[key] = _build(debug)
    return _cached[key]


def _get_runner():
    """Build (once) a jitted 8-core SPMD executable mirroring
    bass2jax.run_bass_via_pjrt, reusable across calls for benchmarking."""
    if "runner" in _cached:
        return _cached["runner"]
    import jax
    import jax.numpy as jnp
    from jax.experimental.shard_map import shard_map
    from jax.sharding import Mesh, PartitionSpec
    import concourse.mybir as mybir
    from concourse import bass2jax

    bass2jax.install_neuronx_cc_hook()
    nc = _get_nc()
    assert nc.dbg_addr is None
    partition_name = nc.partition_id_tensor.name if nc.partition_id_tensor else None

    in_names, out_names, out_avals, zero_outs = [], [], [], []
    for alloc in nc.m.functions[0].allocations:
        if not isinstance(alloc, mybir.MemoryLocationSet):
            continue
        name = alloc.memorylocations[0].name
        if alloc.kind == "ExternalInput":
            if name != partition_name:
                in_names.append(name)
        elif alloc.kind == "ExternalOutput":
            out_names.append(name)
            shape = tuple(alloc.tensor_shape)
            dtype = mybir.dt.np(alloc.dtype)
            out_avals.append(jax.core.ShapedArray(shape, dtype))
            zero_outs.append(np.zeros(shape, dtype))
    n_params = len(in_names)
    all_in_names = in_names + out_names
    if partition_name is not None:
        all_in_names = all_in_names + [partition_name]
    donate = tuple(range(n_params, n_params + len(out_names)))

    def _body(*args):
        operands = list(args)
        if partition_name is not None:
            operands.append(bass2jax.partition_id_tensor())
        outs = bass2jax._bass_exec_p.bind(
            *operands,
            out_avals=tuple(out_avals),
            in_names=tuple(all_in_names),
            out_names=tuple(out_names),
            lowering_input_output_aliases=(),
            sim_require_finite=True,
            sim_require_nnan=True,
            nc=nc,
        )
        return tuple(outs)

    devices = jax.devices()[:NC]
    mesh = Mesh(np.asarray(devices), ("core",))
    nin = n_params + len(out_names)
    sharded = jax.jit(
        shard_map(
            _body,
            mesh=mesh,
            in_specs=(PartitionSpec("core"),) * nin,
            out_specs=(PartitionSpec("core"),) * len(out_names),
            check_rep=False,
        ),
        donate_argnums=donate,
        keep_unused=True,
    )

    def run(in_maps):
        concat_in = [
            np.concatenate([np.asarray(in_maps[c][n]) for c in range(NC)], axis=0)
            for n in in_names
        ]
        concat_zeros = [
            np.zeros((NC * z.shape[0], *z.shape[1:]), z.dtype) for z in zero_outs
        ]
        out_arrs = sharded(*concat_in, *concat_zeros)
        return [
            {
                n: np.asarray(out_arrs[i]).reshape(NC, *out_avals[i].shape)[c]
                for i, n in enumerate(out_names)
            }
            for c in range(NC)
        ]

    _cached["runner"] = (run, sharded, in_names, out_names, out_avals, zero_outs)
    return _cached["runner"]


def _make_in_maps(query, key, value, Wq, bq, Wk, bk, Wv, bv, Wo, bo):

    query = np.asarray(query, dtype=np.float32)
    key = np.asarray(key, dtype=np.float32)
    value = np.asarray(value, dtype=np.float32)
    Wq, Wk, Wv, Wo = (np.asarray(a, dtype=np.float32) for a in (Wq, Wk, Wv, Wo))
    bq, bk, bv, bo = (np.asarray(a, dtype=np.float32) for a in (bq, bk, bv, bo))
    B = query.shape[0]
    ident = np.eye(128, dtype=np.float32)

    xqT = [np.ascontiguousarray(query[b].T) for b in range(B)]
    xkT = [np.ascontiguousarray(key[b].T) for b in range(B)]
    xvT = [np.ascontiguousarray(value[b].T) for b in range(B)]

    in_maps = []
    for c in range(NC):
        b, hg = divmod(c, NC // B)
        sl = slice(hg * M, (hg + 1) * M)
        in_maps.append(
            {
                "xqT": xqT[b],
                "xkT": xkT[b],
                "xvT": xvT[b],
                "wq": np.ascontiguousarray(Wq[:, sl]),
                "wk": np.ascontiguousarray(Wk[:, sl]),
                "wv": np.ascontiguousarray(Wv[:, sl]),
                "wo": np.ascontiguousarray(Wo[sl, :]),
                "bq": np.ascontiguousarray(bq[sl]),
                "bk": np.ascontiguousarray(bk[sl]),
                "bv": np.ascontiguousarray(bv[sl]),
                "ident": ident,
            }
        )
    return in_maps


def kernel(query, key, value, Wq, bq, Wk, bk, Wv, bv, Wo, bo):
    in_maps = _make_in_maps(query, key, value, Wq, bq, Wk, bk, Wv, bv, Wo, bo)
    run = _get_runner()[0]
    results = run(in_maps)

    B = np.asarray(query).shape[0]
    bo = np.asarray(bo, dtype=np.float32)
    full = np.zeros((B, S, D), np.float32)
    for b in range(B):
        acc = np.zeros((S, D), np.float32)
        for g in range(NC // B):
            acc += results[b * (NC // B) + g]["out"]
        full[b] = acc + bo[None, :]
    return full
